# revision 1
# baseline (speedup 1.0000x reference)
"""Causal self-attention (B=4,T=2048,C=1024,H=16,rope) on 8 trn2 cores.

Sharding: core i -> batch b=i>>1, head-group g=i&1 (heads 8g..8g+7).
Device pipeline (feature-major [feat, tok] layouts, all host transposes free):
  QKV^T projections -> rope -> transposed-flash attention (S^T layout,
  no-max softmax, ones-column row sums fused into AV) -> pairwise
  AllGather of attention outputs -> output projection (c_out split
  across the pair; host slices c_proj weights per core).
Output per core: z^T slice [512 c_out, 2048 t]; host reassembles.
"""
import numpy as np
import ml_dtypes

import concourse.bass as bass
import concourse.tile as tile
from concourse import bacc, mybir
from concourse.bass_utils import run_bass_kernel_spmd

F32 = mybir.dt.float32
BF16 = mybir.dt.bfloat16

B, T, C = 4, 2048, 1024
H = 16          # total heads
D = 64          # head dim
GH = 8          # heads per core (group)
GF = GH * D     # 512 features per group
NKC = T // 128  # 16 k-chunks
NQB = T // 512  # 4 q-blocks
NCC = C // 128  # 8 contraction chunks

_NC_CACHE = {}


def _build():
    nc = bacc.Bacc("TRN2", target_bir_lowering=False, debug=False, num_devices=8)
    AF = mybir.ActivationFunctionType

    xt_e = nc.declare_dram_parameter("xt", [C, T], BF16, isOutput=False)
    wq_e = nc.declare_dram_parameter("wq", [C, GF], BF16, isOutput=False)
    wk_e = nc.declare_dram_parameter("wk", [C, GF], BF16, isOutput=False)
    wv_e = nc.declare_dram_parameter("wv", [C, GF], BF16, isOutput=False)
    wp_e = nc.declare_dram_parameter("wp", [C, 512], BF16, isOutput=False)
    cos_e = nc.declare_dram_parameter("cos", [16, T], BF16, isOutput=False)
    sin_e = nc.declare_dram_parameter("sin", [16, T], BF16, isOutput=False)
    msk_e = nc.declare_dram_parameter("msk", [128, 128], F32, isOutput=False)
    one_e = nc.declare_dram_parameter("one", [1, 64], BF16, isOutput=False)
    bia_e = nc.declare_dram_parameter("bia", [128, 4], F32, isOutput=False)
    out_e = nc.declare_dram_parameter("out", [512, T], F32, isOutput=True)

    cc_in = nc.dram_tensor("cc_in", [GF, T], BF16)
    cc_out = nc.dram_tensor("cc_out", [C, T], BF16)

    with tile.TileContext(nc) as tc:
        with tc.tile_pool(name="const", bufs=1) as cpool, \
             tc.tile_pool(name="big", bufs=1) as big, \
             tc.tile_pool(name="work", bufs=2) as work, \
             tc.tile_pool(name="ps_qkv", bufs=2, space="PSUM") as ps_qkv, \
             tc.tile_pool(name="ps_s", bufs=2, space="PSUM") as ps_s, \
             tc.tile_pool(name="ps_av", bufs=1, space="PSUM") as ps_av:

            # ---------------- constants / inputs ----------------
            xt = big.tile([128, NCC, T], BF16, tag="xt")
            for c in range(NCC):
                nc.sync.dma_start(out=xt[:, c, :], in_=xt_e[c * 128:(c + 1) * 128, :])
            wq = cpool.tile([128, NCC, GF], BF16)
            wk = cpool.tile([128, NCC, GF], BF16)
            wv = cpool.tile([128, NCC, GF], BF16)
            wp = cpool.tile([128, NCC, 512], BF16)
            for c in range(NCC):
                nc.sync.dma_start(out=wq[:, c, :], in_=wq_e[c * 128:(c + 1) * 128, :])
                nc.sync.dma_start(out=wk[:, c, :], in_=wk_e[c * 128:(c + 1) * 128, :])
                nc.sync.dma_start(out=wv[:, c, :], in_=wv_e[c * 128:(c + 1) * 128, :])
                nc.sync.dma_start(out=wp[:, c, :], in_=wp_e[c * 128:(c + 1) * 128, :])
            cos = cpool.tile([16, T], BF16)
            sin = cpool.tile([16, T], BF16)
            msk = cpool.tile([128, 128], F32)
            one = cpool.tile([1, 64], BF16)
            bia = cpool.tile([128, 4], F32)
            nc.sync.dma_start(out=cos, in_=cos_e[:, :])
            nc.sync.dma_start(out=sin, in_=sin_e[:, :])
            nc.sync.dma_start(out=msk, in_=msk_e[:, :])
            nc.sync.dma_start(out=one, in_=one_e[:, :])
            nc.sync.dma_start(out=bia, in_=bia_e[:, :])

            qt = big.tile([128, 4, T], BF16, tag="qt")   # [hp-block rows, hp, t]
            kt = big.tile([128, 4, T], BF16, tag="kt")
            va = big.tile([128, NKC, GH, 65], BF16, tag="va")  # V natural + ones col
            ot = big.tile([128, 4, T], BF16, tag="ot")   # attention out^T
            nc.vector.memset(va[:, :, :, 64:65], 1.0)

            # ---------------- QKV projections + rope ----------------
            def qkv_proj(wt, dst, rope):
                for hp in range(4):
                    for tb in range(NQB):
                        ps = ps_qkv.tile([128, 512], F32, tag="qkv")
                        for c in range(NCC):
                            nc.tensor.matmul(
                                ps, lhsT=wt[:, c, hp * 128:(hp + 1) * 128],
                                rhs=xt[:, c, tb * 512:(tb + 1) * 512],
                                start=(c == 0), stop=(c == NCC - 1))
                        d = dst[:, hp, tb * 512:(tb + 1) * 512]
                        if not rope:
                            nc.vector.tensor_copy(d, ps)
                            continue
                        # full cast, then overwrite rope rows (d 0:16, 32:48 per head)
                        nc.vector.tensor_copy(d, ps)
                        cs = cos[:, tb * 512:(tb + 1) * 512]
                        sn = sin[:, tb * 512:(tb + 1) * 512]
                        for ho in (0, 64):
                            x1 = ps[ho:ho + 16, :]
                            x2 = ps[ho + 32:ho + 48, :]
                            t1 = work.tile([16, 512], F32, tag="r1")
                            t2 = work.tile([16, 512], F32, tag="r2")
                            nc.vector.tensor_tensor(out=t1, in0=x1, in1=cs,
                                                    op=mybir.AluOpType.mult)
                            nc.vector.tensor_tensor(out=t2, in0=x2, in1=sn,
                                                    op=mybir.AluOpType.mult)
                            nc.vector.tensor_tensor(
                                out=dst[ho:ho + 16, hp, tb * 512:(tb + 1) * 512],
                                in0=t1, in1=t2, op=mybir.AluOpType.add)
                            t3 = work.tile([16, 512], F32, tag="r3")
                            t4 = work.tile([16, 512], F32, tag="r4")
                            nc.vector.tensor_tensor(out=t3, in0=x2, in1=cs,
                                                    op=mybir.AluOpType.mult)
                            nc.vector.tensor_tensor(out=t4, in0=x1, in1=sn,
                                                    op=mybir.AluOpType.mult)
                            nc.vector.tensor_tensor(
                                out=dst[ho + 32:ho + 48, hp, tb * 512:(tb + 1) * 512],
                                in0=t3, in1=t4, op=mybir.AluOpType.subtract)

            qkv_proj(wq, qt, rope=True)
            qkv_proj(wk, kt, rope=True)
            # V natural [k-token rows, group features]
            for kc in range(NKC):
                ps = ps_qkv.tile([128, 512], F32, tag="qkv")
                for c in range(NCC):
                    nc.tensor.matmul(
                        ps, lhsT=xt[:, c, kc * 128:(kc + 1) * 128],
                        rhs=wv[:, c, :], start=(c == 0), stop=(c == NCC - 1))
                nc.vector.tensor_copy(va[:, kc, :, 0:64], ps.rearrange("p (h d) -> p h d", h=GH))

            # ---------------- attention (S^T flash, no-max softmax) ------------
            for h in range(GH):
                hp, ro = h >> 1, (h & 1) * 64
                avs = []
                for j in range(NQB):
                    avt = ps_av.tile([65, 512], F32, tag=f"av{j}", name=f"av_{h}_{j}")
                    avs.append(avt)
                for kc in range(NKC):
                    jmin = kc // 4
                    for j in range(jmin, NQB):
                        off = 128 * (kc % 4) if j == jmin else 0
                        w = 512 - off
                        sp = ps_s.tile([128, 512], F32, tag="s")
                        nc.tensor.matmul(
                            sp[:, 0:w],
                            lhsT=kt[ro:ro + 64, hp, kc * 128:(kc + 1) * 128],
                            rhs=qt[ro:ro + 64, hp, j * 512 + off:(j + 1) * 512],
                            start=True, stop=True)
                        if j == jmin:
                            nc.vector.tensor_tensor(out=sp[:, 0:128], in0=sp[:, 0:128],
                                                    in1=msk, op=mybir.AluOpType.add)
                        pb = work.tile([128, 512], BF16, tag="p")
                        nc.scalar.activation(out=pb[:, 0:w], in_=sp[:, 0:w],
                                             func=AF.Exp, scale=0.125)
                        nc.tensor.matmul(
                            avs[j][:, off:512], lhsT=va[:, kc, h, :],
                            rhs=pb[:, 0:w], start=(kc == 0), stop=(kc == 4 * j + 3))
                for j in range(NQB):
                    rc = work.tile([1, 512], F32, tag="rc")
                    nc.vector.reciprocal(rc, avs[j][64:65, :])
                    rb = work.tile([1, 512], BF16, tag="rb")
                    nc.vector.tensor_copy(rb, rc)
                    bc = ps_s.tile([64, 512], F32, tag="s")
                    nc.tensor.matmul(bc, lhsT=one, rhs=rb, start=True, stop=True)
                    bs = work.tile([64, 512], BF16, tag="bs")
                    nc.vector.tensor_copy(bs, bc)
                    nc.vector.tensor_tensor(
                        out=ot[ro:ro + 64, hp, j * 512:(j + 1) * 512],
                        in0=avs[j][0:64, :], in1=bs, op=mybir.AluOpType.mult)

            # ---------------- exchange + output projection ----------------
            for fb in range(4):
                nc.sync.dma_start(out=cc_in[fb * 128:(fb + 1) * 128, :],
                                  in_=ot[:, fb, :])
            nc.gpsimd.collective_compute(
                "AllGather", mybir.AluOpType.bypass,
                replica_groups=[[0, 1], [2, 3], [4, 5], [6, 7]],
                ins=[cc_in[:, :]], outs=[cc_out[:, :]])
            og = big.tile([128, NCC, T], BF16, tag="xt")
            for c in range(NCC):
                nc.sync.dma_start(out=og[:, c, :], in_=cc_out[c * 128:(c + 1) * 128, :])
            for tb in range(NQB):
                for cb in range(4):
                    zp = ps_qkv.tile([128, 512], F32, tag="qkv")
                    for c in range(NCC):
                        nc.tensor.matmul(
                            zp, lhsT=wp[:, c, cb * 128:(cb + 1) * 128],
                            rhs=og[:, c, tb * 512:(tb + 1) * 512],
                            start=(c == 0), stop=(c == NCC - 1))
                    zs = work.tile([128, 512], F32, tag="z")
                    nc.vector.tensor_scalar_add(zs, zp, bia[:, cb:cb + 1])
                    nc.sync.dma_start(
                        out=out_e[cb * 128:(cb + 1) * 128, tb * 512:(tb + 1) * 512],
                        in_=zs)
    nc.compile()
    return nc


def _prep(x, qkv_w, c_proj_w, c_proj_b):
    bf16 = ml_dtypes.bfloat16
    af16 = (1.0 / 1024.0) ** np.linspace(0.0, 1.0, 16, dtype=np.float32)
    th = np.arange(T, dtype=np.float32)[None, :] * af16[:, None]  # [16, T]
    cos, sin = np.cos(th).astype(np.float32), np.sin(th).astype(np.float32)
    kl = np.arange(128)[:, None]
    ql = np.arange(128)[None, :]
    msk = np.where(kl <= ql, 0.0, -60000.0).astype(np.float32)
    one = np.ones((1, 64), bf16)
    maps = []
    for i in range(8):
        b, g = i >> 1, i & 1
        gs = slice(g * GF, (g + 1) * GF)
        cs = slice(g * 512, (g + 1) * 512)
        bia = np.ascontiguousarray(
            c_proj_b[cs].reshape(4, 128).T.astype(np.float32))
        maps.append({
            "xt": np.ascontiguousarray(x[b].T).astype(bf16),
            "wq": np.ascontiguousarray(qkv_w[0][gs, :].T).astype(bf16),
            "wk": np.ascontiguousarray(qkv_w[1][gs, :].T).astype(bf16),
            "wv": np.ascontiguousarray(qkv_w[2][gs, :].T).astype(bf16),
            "wp": np.ascontiguousarray(c_proj_w[cs, :].T).astype(bf16),
            "cos": cos.astype(bf16), "sin": sin.astype(bf16), "msk": msk, "one": one, "bia": bia,
        })
    return maps


def kernel(x, qkv_w, c_proj_w, c_proj_b, _want_time=False):
    key = "nc"
    if key not in _NC_CACHE:
        _NC_CACHE[key] = _build()
    nc = _NC_CACHE[key]
    maps = _prep(np.asarray(x), np.asarray(qkv_w), np.asarray(c_proj_w),
                 np.asarray(c_proj_b))
    import time
    res = run_bass_kernel_spmd(nc, maps, core_ids=list(range(8)))
    t_ns = None
    if _want_time:
        t0 = time.perf_counter()
        res = run_bass_kernel_spmd(nc, maps, core_ids=list(range(8)))
        t_ns = int((time.perf_counter() - t0) * 1e9)
    out = np.empty((B, T, C), np.float32)
    for i in range(8):
        b, g = i >> 1, i & 1
        out[b, :, g * 512:(g + 1) * 512] = res.results[i]["out"].T
    if _want_time:
        return out, t_ns
    return out

